# revision 1
# baseline (speedup 1.0000x reference)
"""GAT (2-layer, DGL GATConv semantics) on 8 Trainium2 NeuronCores.

Strategy (dst-owner sharding, two launches):
  - Nodes are split into 8 contiguous ranges of 2500; core c owns the dst
    nodes in range c and all edges whose dst it owns.
  - Launch 1 (per core): replicated node GEMM builds the full node table
    T1[n] = [feat1(n)+b1 | el1(n) | er1(n)] (bf16, 384-col rows = 768B) in
    local DRAM; then a dst-major edge phase (dst nodes on partitions,
    incoming edges along the free dim, degree-sorted binning into 128-row
    blocks) computes h1 = GATConv1 for owned dsts via dma_gather of source
    rows; then elu + local GEMM2 produce T2_own = [feat2+b2 | el2 | er2].
  - Host concatenates T2_own from all cores into T2_full (the "all-gather";
    a device collective was measured at ~90us fixed - slower than reloading).
  - Launch 2 (per core): same edge phase against T2_full (256B rows)
    produces the final [2500, 64] rows, which the host un-permutes.

Edge softmax: computed without the segment-max subtraction (logits are
~+-10 here, exp() is safe in f32; alpha is mathematically identical), and
the division by the denominator happens after aggregation (it is constant
within a dst segment).  The bias is folded into the table rows (sum alpha
= 1).  Padding slots point at a sentinel row with el = -200 (weight
exp(0.2*-200) ~ 4e-18); isolated dsts point all slots at a row
[bias | el=+80] so their output is exactly the bias.
"""

import numpy as np
import ml_dtypes

import concourse.bass as bass
import concourse.bacc as bacc
import concourse.tile as tile
import concourse.mybir as mybir
from concourse import bass_utils

BF16 = mybir.dt.bfloat16
F32 = mybir.dt.float32
I16 = mybir.dt.int16

N_NODES = 20000
N_EDGES = 320000
IN_FEATS = 256
H1, O1 = 4, 64
H2, O2 = 1, 64
NEG_SLOPE = 0.2
N_CORES = 8
OWN = N_NODES // N_CORES          # 2500 dst nodes per core
NBLK = (OWN + 127) // 128         # 20 blocks of 128 dst nodes
RANKS = NBLK * 128                # 2560 ranks (last 60 dead)
NQ = 4                            # SWDGE queues for gathers
JSLICE = 8                        # gather sub-call: 8*128 = 1024 rows

T1_COLS = 384                     # feat(256) el(4) er(4) pad -> 768B rows
T1_FEAT = H1 * O1                 # 256
T1_ROWS_GEMM = ((N_NODES + 1023) // 1024) * 1024  # 20480 (8-tile groups)
T1_PAD = T1_ROWS_GEMM
T1_ISO = T1_ROWS_GEMM + 1
T1_ROWS = T1_ROWS_GEMM + 2

T2_COLS = 128                     # feat2(64) el2(1) er2(1) pad -> 256B rows
T2_FEAT = H2 * O2
T2_PAD = N_CORES * OWN
T2_ISO = T2_PAD + 1
T2_ROWS = T2_PAD + 2

EL_BIG = 80.0
EL_SMALL = -200.0

PROFILE = False          # test.py sets True to collect exec_time_ns
LAST_EXEC_NS = None      # [launch1_ns, launch2_ns] when PROFILE


def _bf(x):
    return np.asarray(x, dtype=ml_dtypes.bfloat16)


# ----------------------------------------------------------------------------
# host-side preprocessing
# ----------------------------------------------------------------------------

def t1row_of(n):
    n = np.asarray(n)
    t = n // 128
    return (t // 8) * 1024 + (n % 128) * 8 + (t % 8)


def preprocess(src, dst):
    src = np.asarray(src).astype(np.int64)
    dst = np.asarray(dst).astype(np.int64)
    owner = dst // OWN
    cores = []
    for c in range(N_CORES):
        sel = np.nonzero(owner == c)[0]
        dloc = dst[sel] - c * OWN
        deg = np.bincount(dloc, minlength=OWN)
        order = np.argsort(deg, kind="stable")
        rank_of = np.empty(OWN, dtype=np.int64)
        rank_of[order] = np.arange(OWN)
        cores.append(dict(sel=sel, dloc=dloc, deg=deg, order=order, rank_of=rank_of))
    t2row_of = np.empty(N_NODES, dtype=np.int64)
    for c in range(N_CORES):
        t2row_of[c * OWN + cores[c]["order"]] = c * OWN + np.arange(OWN)

    # global per-block J (slots per dst) so the SPMD program is uniform
    J = []
    degsorted = [np.sort(cores[c]["deg"]) for c in range(N_CORES)]
    for b in range(NBLK):
        jb = max(int(degsorted[c][b * 128:(b + 1) * 128].max(initial=0))
                 for c in range(N_CORES))
        J.append(max(1, jb))

    for c in range(N_CORES):
        d = cores[c]
        eorder = np.lexsort((src[d["sel"]], d["rank_of"][d["dloc"]]))
        es = d["sel"][eorder]
        eranks = d["rank_of"][d["dloc"][eorder]]
        starts = np.searchsorted(eranks, np.arange(RANKS))
        degs_by_rank = np.searchsorted(eranks, np.arange(RANKS), side="right") - starts
        idx1_blocks, idx2_blocks = [], []
        for b in range(NBLK):
            jb = J[b]
            i1 = np.full((jb, 128), T1_PAD, dtype=np.int32)
            i2 = np.full((jb, 128), T2_PAD, dtype=np.int32)
            for p in range(128):
                r = b * 128 + p
                if r >= OWN:
                    continue
                k = int(degs_by_rank[r])
                if k == 0:
                    i1[:, p] = T1_ISO
                    i2[:, p] = T2_ISO
                    continue
                s_nodes = src[es[starts[r]:starts[r] + k]]
                i1[:k, p] = t1row_of(s_nodes)
                i2[:k, p] = t2row_of[s_nodes]
            idx1_blocks.append(i1)
            idx2_blocks.append(i2)
        er1 = np.full(RANKS, T1_PAD, dtype=np.int32)
        er2 = np.full(RANKS, T2_PAD, dtype=np.int32)
        er1[:OWN] = t1row_of(c * OWN + d["order"])
        er2[:OWN] = c * OWN + np.arange(OWN)
        d.update(idx1_blocks=idx1_blocks, idx2_blocks=idx2_blocks, er1=er1, er2=er2)
    return cores, J


def wrap_idx16(flat):
    """int16 wrap for dma_gather: value i at [i%16, i//16], replicated into
    all 8 16-partition groups so any SWDGE queue's Q7 pair can read it."""
    n = len(flat)
    assert n % 16 == 0
    s = n // 16
    a = np.zeros((128, s), dtype=np.int16)
    ii = np.arange(n)
    a[ii % 16, ii // 16] = flat.astype(np.int16)
    for r in range(1, 8):
        a[16 * r:16 * (r + 1)] = a[0:16]
    return a


def build_idx_cols(idx_blocks):
    chunks, calls = [], []
    col = 0
    for i_b in idx_blocks:
        jb = i_b.shape[0]
        blk_calls = []
        for j0 in range(0, jb, JSLICE):
            js = min(JSLICE, jb - j0)
            w = wrap_idx16(i_b[j0:j0 + js].reshape(-1))
            chunks.append(w)
            blk_calls.append((col, j0, js, js * 128, w.shape[1]))
            col += w.shape[1]
        calls.append(blk_calls)
    return np.concatenate(chunks, axis=1), calls


def er_idx_cols(er_flat):
    chunks, calls = [], []
    col = 0
    for s0 in range(0, RANKS, JSLICE * 128):
        n = min(JSLICE * 128, RANKS - s0)
        w = wrap_idx16(er_flat[s0:s0 + n])
        chunks.append(w)
        calls.append((col, s0 // 128, n // 128, n, w.shape[1]))
        col += w.shape[1]
    return np.concatenate(chunks, axis=1), calls


# ----------------------------------------------------------------------------
# device kernel builders
# ----------------------------------------------------------------------------

class QueueRR:
    def __init__(self):
        self.i = 0

    def __call__(self):
        q = self.i % NQ
        self.i += 1
        return q


def emit_edge_phase(nc, pools, t_ap, t_er_ap, idx_sb, idx_calls,
                    er_idx_sb, er_sb, er_calls, er_col, J, heads, osz, cw,
                    el_off, jmax, out_cb, qrr):
    gpool, mpool, rpool, spool = pools
    fw = heads * osz
    for (col, b0, nb, rows, scols) in er_calls:
        nc.gpsimd.dma_gather(
            out_ap=er_sb[:, b0:b0 + nb, :],
            in_ap=t_er_ap,
            idxs_ap=er_idx_sb[:, col:col + scols],
            num_idxs=rows, num_idxs_reg=rows,
            elem_size=128, elem_step=t_er_ap.ap[0][0],
            queue_num=qrr(),
        )
    for b in range(NBLK):
        jb = J[b]
        G = gpool.tile([128, jmax, cw], BF16, tag="G")
        for (col, j0, js, rows, scols) in idx_calls[b]:
            nc.gpsimd.dma_gather(
                out_ap=G[:, j0:j0 + js, :],
                in_ap=t_ap,
                idxs_ap=idx_sb[:, col:col + scols],
                num_idxs=rows, num_idxs_reg=rows,
                elem_size=cw, elem_step=t_ap.ap[0][0],
                queue_num=qrr(),
            )
        w = spool.tile([128, jmax, heads], F32, tag="w")
        nc.vector.tensor_tensor(
            out=w[:, :jb, :],
            in0=G[:, :jb, el_off:el_off + heads],
            in1=er_sb[:, b:b + 1, er_col:er_col + heads].to_broadcast(
                [128, jb, heads]),
            op=mybir.AluOpType.add,
        )
        wl = spool.tile([128, jmax, heads], F32, tag="wl")
        nc.vector.tensor_scalar(out=wl[:, :jb, :], in0=w[:, :jb, :],
                                scalar1=NEG_SLOPE, scalar2=None,
                                op0=mybir.AluOpType.mult)
        nc.vector.tensor_tensor(out=w[:, :jb, :], in0=w[:, :jb, :],
                                in1=wl[:, :jb, :], op=mybir.AluOpType.max)
        M = mpool.tile([128, jmax, fw + heads], BF16, tag="M")
        nc.scalar.activation(M[:, :jb, fw:fw + heads], w[:, :jb, :],
                             mybir.ActivationFunctionType.Exp)
        for h in range(heads):
            nc.vector.tensor_tensor(
                out=M[:, :jb, h * osz:(h + 1) * osz],
                in0=G[:, :jb, h * osz:(h + 1) * osz],
                in1=M[:, :jb, fw + h:fw + h + 1].to_broadcast([128, jb, osz]),
                op=mybir.AluOpType.mult,
            )
        R = rpool.tile([128, max(1, (jmax + 1) // 2), fw + heads], F32, tag="R")
        k = jb // 2
        if k > 0:
            nc.vector.tensor_tensor(out=R[:, :k, :], in0=M[:, 0:k, :],
                                    in1=M[:, k:2 * k, :], op=mybir.AluOpType.add)
        if jb % 2 == 1:
            nc.vector.tensor_copy(out=R[:, k:k + 1, :], in_=M[:, 2 * k:jb, :])
            k += 1
        while k > 1:
            m = k // 2
            nc.vector.tensor_tensor(out=R[:, :m, :], in0=R[:, 0:m, :],
                                    in1=R[:, m:2 * m, :], op=mybir.AluOpType.add)
            if k % 2 == 1:
                nc.vector.tensor_tensor(out=R[:, 0:1, :], in0=R[:, 0:1, :],
                                        in1=R[:, 2 * m:k, :], op=mybir.AluOpType.add)
            k = m
        rden = spool.tile([128, heads], F32, tag="rden")
        nc.vector.reciprocal(rden[:], R[:, 0, fw:fw + heads])
        rst = spool.tile([128, fw], F32, tag="rst")
        nc.vector.tensor_tensor(
            out=rst[:].rearrange("p (h o) -> p h o", h=heads),
            in0=R[:, 0, 0:fw].rearrange("p (h o) -> p h o", h=heads),
            in1=rden[:].to_broadcast([128, heads, osz]),
            op=mybir.AluOpType.mult,
        )
        out_cb(b, rst)


def build_launch1(J, s_idx, s_er, idx_calls, er_calls):
    nc = bacc.Bacc("TRN2", target_bir_lowering=False, debug=False,
                   num_devices=N_CORES, num_swdge_queues=NQ)
    ntile = T1_ROWS_GEMM // 128
    ngrp = (ntile + 7) // 8
    xT = nc.dram_tensor("xT", [ngrp, 128, 8, 2, 128], BF16, kind="ExternalInput")
    w1pad = nc.dram_tensor("w1pad", [IN_FEATS, 264], BF16, kind="ExternalInput")
    w1T = nc.dram_tensor("w1T", [IN_FEATS, IN_FEATS], BF16, kind="ExternalInput")
    albd1 = nc.dram_tensor("albd1", [IN_FEATS, 8], BF16, kind="ExternalInput")
    b1bc = nc.dram_tensor("b1bc", [128, 264], BF16, kind="ExternalInput")
    w2pad = nc.dram_tensor("w2pad", [T1_FEAT, 66], BF16, kind="ExternalInput")
    w2T = nc.dram_tensor("w2T", [O2, T1_FEAT], BF16, kind="ExternalInput")
    albd2 = nc.dram_tensor("albd2", [O2, 2], BF16, kind="ExternalInput")
    b2bc = nc.dram_tensor("b2bc", [128, 66], BF16, kind="ExternalInput")
    padiso = nc.dram_tensor("padiso", [2, T1_COLS], BF16, kind="ExternalInput")
    identw = nc.dram_tensor("identw", [128, 128], BF16, kind="ExternalInput")
    idx1 = nc.dram_tensor("idx1", [128, s_idx], I16, kind="ExternalInput")
    eridx1 = nc.dram_tensor("eridx1", [128, s_er], I16, kind="ExternalInput")
    t2own = nc.dram_tensor("t2own", [NBLK, 128, 66], BF16, kind="ExternalOutput")
    T1 = nc.dram_tensor("T1", [T1_ROWS, T1_COLS], BF16, kind="Internal")

    jmax = max(J)
    qrr = QueueRR()
    with tile.TileContext(nc) as tc:
        with (
            tc.tile_pool(name="const", bufs=1) as cpool,
            tc.tile_pool(name="xin", bufs=4) as xpool,
            tc.tile_pool(name="psA", bufs=2, space="PSUM") as ppa,
            tc.tile_pool(name="psB", bufs=2, space="PSUM") as ppb,
            tc.tile_pool(name="tout", bufs=4) as tpool,
            tc.tile_pool(name="G", bufs=2) as gpool,
            tc.tile_pool(name="M", bufs=2) as mpool,
            tc.tile_pool(name="R", bufs=2) as rpool,
            tc.tile_pool(name="small", bufs=3) as spool,
            tc.tile_pool(name="h1", bufs=2) as hpool,
        ):
            ident = cpool.tile([128, 128], BF16)
            nc.sync.dma_start(ident[:], identw.ap())
            idx_sb = cpool.tile([128, s_idx], I16)
            nc.sync.dma_start(idx_sb[:], idx1.ap())
            eridx_sb = cpool.tile([128, s_er], I16)
            nc.sync.dma_start(eridx_sb[:], eridx1.ap())
            er_sb = cpool.tile([128, NBLK, 128], BF16)

            # --- W1ext = [W1 | W1@albd1] as two K-chunks [128, 264] ---
            w1ext = [cpool.tile([128, 264], BF16, tag=f"w1e{k}", name=f"w1e{k}")
                     for k in (0, 1)]
            for k in (0, 1):
                nc.sync.dma_start(w1ext[k][:], w1pad.ap()[k * 128:(k + 1) * 128, :])
            w1T_sb = cpool.tile([128, 2, 2, 128], BF16)
            for ck in (0, 1):
                for mk in (0, 1):
                    nc.sync.dma_start(
                        w1T_sb[:, ck, mk, :],
                        w1T.ap()[ck * 128:(ck + 1) * 128, mk * 128:(mk + 1) * 128])
            albd1_sb = cpool.tile([128, 2, 8], BF16)
            for ck in (0, 1):
                nc.sync.dma_start(albd1_sb[:, ck, :],
                                  albd1.ap()[ck * 128:(ck + 1) * 128, :])
            for mk in (0, 1):
                ps = ppb.tile([128, 8], F32, tag="aux")
                nc.tensor.matmul(ps[:], lhsT=w1T_sb[:, 0, mk, :],
                                 rhs=albd1_sb[:, 0, :], start=True, stop=False)
                nc.tensor.matmul(ps[:], lhsT=w1T_sb[:, 1, mk, :],
                                 rhs=albd1_sb[:, 1, :], start=False, stop=True)
                nc.vector.tensor_copy(out=w1ext[mk][:, 256:264], in_=ps[:])
            b1bc_sb = cpool.tile([128, 264], BF16)
            nc.sync.dma_start(b1bc_sb[:], b1bc.ap())

            # --- W2ext = [W2 | W2@albd2] ---
            w2ext = [cpool.tile([128, 66], BF16, tag=f"w2e{k}", name=f"w2e{k}")
                     for k in (0, 1)]
            for k in (0, 1):
                nc.sync.dma_start(w2ext[k][:], w2pad.ap()[k * 128:(k + 1) * 128, :])
            w2T_sb = cpool.tile([64, 2, 128], BF16)
            for mk in (0, 1):
                nc.sync.dma_start(w2T_sb[:, mk, :],
                                  w2T.ap()[:, mk * 128:(mk + 1) * 128])
            albd2_sb = cpool.tile([64, 2], BF16)
            nc.sync.dma_start(albd2_sb[:], albd2.ap())
            for mk in (0, 1):
                ps = ppb.tile([128, 2], F32, tag="aux")
                nc.tensor.matmul(ps[:], lhsT=w2T_sb[:, mk, :], rhs=albd2_sb[:],
                                 start=True, stop=True)
                nc.vector.tensor_copy(out=w2ext[mk][:, 64:66], in_=ps[:])
            b2bc_sb = cpool.tile([128, 66], BF16)
            nc.sync.dma_start(b2bc_sb[:], b2bc.ap())

            # --- replicated GEMM over all nodes -> T1 (groups of 8 tiles) ---
            for g in range(ngrp):
                xsb = xpool.tile([128, 8, 2, 128], BF16, tag="x")
                nc.sync.dma_start(xsb[:], xT.ap()[g])
                tsb = tpool.tile([128, 8, T1_COLS], BF16, tag="t1")
                nc.vector.memset(tsb[:, :, 264:T1_COLS], 0.0)
                for i in range(8):
                    ps = ppa.tile([128, 264], F32, tag="gemm1")
                    nc.tensor.matmul(ps[:], lhsT=xsb[:, i, 0, :], rhs=w1ext[0][:],
                                     start=True, stop=False)
                    nc.tensor.matmul(ps[:], lhsT=xsb[:, i, 1, :], rhs=w1ext[1][:],
                                     start=False, stop=True)
                    nc.vector.tensor_tensor(out=tsb[:, i, 0:264], in0=ps[:],
                                            in1=b1bc_sb[:], op=mybir.AluOpType.add)
                nc.scalar.dma_start(
                    T1.ap()[g * 1024:(g + 1) * 1024, :].rearrange(
                        "(p i) c -> p i c", i=8),
                    tsb[:])
            nc.scalar.dma_start(T1.ap()[T1_ROWS_GEMM:T1_ROWS, :], padiso.ap())

            def out_cb(b, rst):
                p1 = spool.tile([128, T1_FEAT], F32, tag="p1")
                nc.vector.tensor_scalar(out=p1[:], in0=rst[:], scalar1=0.0,
                                        scalar2=1.0, op0=mybir.AluOpType.max,
                                        op1=mybir.AluOpType.subtract)
                nn = spool.tile([128, T1_FEAT], F32, tag="nn")
                nc.vector.tensor_scalar(out=nn[:], in0=rst[:], scalar1=0.0,
                                        scalar2=None, op0=mybir.AluOpType.min)
                nc.scalar.activation(nn[:], nn[:], mybir.ActivationFunctionType.Exp)
                h1b = hpool.tile([128, T1_FEAT], BF16, tag="h1")
                nc.vector.tensor_tensor(out=h1b[:], in0=p1[:], in1=nn[:],
                                        op=mybir.AluOpType.add)
                hT = hpool.tile([128, 2, 128], BF16, tag="hT")
                for k in (0, 1):
                    tp = ppb.tile([128, 128], BF16, tag="tp")
                    nc.tensor.transpose(tp[:], h1b[:, k * 128:(k + 1) * 128], ident[:])
                    nc.vector.tensor_copy(out=hT[:, k, :], in_=tp[:])
                ps2 = ppb.tile([128, 66], F32, tag="aux")
                nc.tensor.matmul(ps2[:], lhsT=hT[:, 0, :], rhs=w2ext[0][:],
                                 start=True, stop=False)
                nc.tensor.matmul(ps2[:], lhsT=hT[:, 1, :], rhs=w2ext[1][:],
                                 start=False, stop=True)
                t2sb = tpool.tile([128, 66], BF16, tag="t2")
                nc.vector.tensor_tensor(out=t2sb[:], in0=ps2[:],
                                        in1=b2bc_sb[:], op=mybir.AluOpType.add)
                nc.sync.dma_start(t2own.ap()[b], t2sb[:])

            emit_edge_phase(
                nc, (gpool, mpool, rpool, spool),
                t_ap=T1.ap(), t_er_ap=T1.ap()[:, 256:T1_COLS],
                idx_sb=idx_sb, idx_calls=idx_calls,
                er_idx_sb=eridx_sb, er_sb=er_sb, er_calls=er_calls,
                er_col=4, J=J, heads=H1, osz=O1, cw=T1_COLS, el_off=256,
                jmax=jmax, out_cb=out_cb, qrr=qrr)
    nc.compile()
    return nc


def build_launch2(J, s_idx, s_er, idx_calls, er_calls):
    nc = bacc.Bacc("TRN2", target_bir_lowering=False, debug=False,
                   num_devices=N_CORES, num_swdge_queues=NQ)
    T2 = nc.dram_tensor("T2", [T2_ROWS, T2_COLS], BF16, kind="ExternalInput")
    idx2 = nc.dram_tensor("idx2", [128, s_idx], I16, kind="ExternalInput")
    eridx2 = nc.dram_tensor("eridx2", [128, s_er], I16, kind="ExternalInput")
    out = nc.dram_tensor("out", [NBLK, 128, O2], F32, kind="ExternalOutput")

    jmax = max(J)
    qrr = QueueRR()
    with tile.TileContext(nc) as tc:
        with (
            tc.tile_pool(name="const", bufs=1) as cpool,
            tc.tile_pool(name="G", bufs=2) as gpool,
            tc.tile_pool(name="M", bufs=2) as mpool,
            tc.tile_pool(name="R", bufs=2) as rpool,
            tc.tile_pool(name="small", bufs=3) as spool,
        ):
            idx_sb = cpool.tile([128, s_idx], I16)
            nc.sync.dma_start(idx_sb[:], idx2.ap())
            eridx_sb = cpool.tile([128, s_er], I16)
            nc.sync.dma_start(eridx_sb[:], eridx2.ap())
            er_sb = cpool.tile([128, NBLK, 128], BF16)

            def out_cb(b, rst):
                nc.sync.dma_start(out.ap()[b], rst[:])

            emit_edge_phase(
                nc, (gpool, mpool, rpool, spool),
                t_ap=T2.ap(), t_er_ap=T2.ap(),
                idx_sb=idx_sb, idx_calls=idx_calls,
                er_idx_sb=eridx_sb, er_sb=er_sb, er_calls=er_calls,
                er_col=65, J=J, heads=H2, osz=O2, cw=T2_COLS, el_off=64,
                jmax=jmax, out_cb=out_cb, qrr=qrr)
    nc.compile()
    return nc


# ----------------------------------------------------------------------------
# host glue
# ----------------------------------------------------------------------------

def _host_weights(W1, al1, ar1, b1, W2, al2, ar2, b2):
    w1pad = np.zeros((IN_FEATS, 264), np.float32)
    w1pad[:, 0:256] = W1
    albd1 = np.zeros((IN_FEATS, 8), np.float32)
    for h in range(H1):
        albd1[h * O1:(h + 1) * O1, h] = al1[h]
        albd1[h * O1:(h + 1) * O1, 4 + h] = ar1[h]
    b1e = np.zeros((128, 264), np.float32)
    b1e[:, 0:256] = b1
    w2pad = np.zeros((T1_FEAT, 66), np.float32)
    w2pad[:, 0:64] = W2
    albd2 = np.zeros((O2, 2), np.float32)
    albd2[:, 0] = al2[0]
    albd2[:, 1] = ar2[0]
    b2e = np.zeros((128, 66), np.float32)
    b2e[:, 0:64] = b2
    padiso1 = np.zeros((2, T1_COLS), np.float32)
    padiso1[0, 256:260] = EL_SMALL
    padiso1[1, 0:256] = b1
    padiso1[1, 256:260] = EL_BIG
    return dict(w1pad=_bf(w1pad), w1T=_bf(np.ascontiguousarray(W1.T)),
                albd1=_bf(albd1), b1bc=_bf(b1e), w2pad=_bf(w2pad),
                w2T=_bf(np.ascontiguousarray(W2.T)), albd2=_bf(albd2),
                b2bc=_bf(b2e), padiso=_bf(padiso1),
                identw=_bf(np.eye(128, dtype=np.float32)))


def kernel(in_feat, W1, al1, ar1, b1, W2, al2, ar2, b2, src, dst):
    in_feat = np.asarray(in_feat, np.float32)
    W1 = np.asarray(W1, np.float32); W2 = np.asarray(W2, np.float32)
    al1 = np.asarray(al1, np.float32); ar1 = np.asarray(ar1, np.float32)
    al2 = np.asarray(al2, np.float32); ar2 = np.asarray(ar2, np.float32)
    b1 = np.asarray(b1, np.float32); b2 = np.asarray(b2, np.float32)

    cores, J = preprocess(src, dst)
    idx1_cols, idx2_cols, er1_cols, er2_cols = [], [], [], []
    calls1 = calls2 = ecalls1 = ecalls2 = None
    for c in range(N_CORES):
        a1, calls1 = build_idx_cols(cores[c]["idx1_blocks"])
        a2, calls2 = build_idx_cols(cores[c]["idx2_blocks"])
        e1, ecalls1 = er_idx_cols(cores[c]["er1"])
        e2, ecalls2 = er_idx_cols(cores[c]["er2"])
        idx1_cols.append(a1); idx2_cols.append(a2)
        er1_cols.append(e1); er2_cols.append(e2)

    wts = _host_weights(W1, al1, ar1, b1, W2, al2, ar2, b2)
    ntile = T1_ROWS_GEMM // 128
    ngrp = (ntile + 7) // 8
    xT = np.zeros((IN_FEATS, ngrp * 1024), np.float32)
    xT[:, :N_NODES] = in_feat.T
    xT = _bf(np.ascontiguousarray(
        xT.reshape(2, 128, ngrp, 8, 128).transpose(2, 1, 3, 0, 4)))

    nc1 = build_launch1(J, idx1_cols[0].shape[1], er1_cols[0].shape[1],
                        calls1, ecalls1)
    in_maps1 = []
    for c in range(N_CORES):
        m = dict(wts)
        m["xT"] = xT
        m["idx1"] = idx1_cols[c]
        m["eridx1"] = er1_cols[c]
        in_maps1.append(m)
    res1 = bass_utils.run_bass_kernel_spmd(nc1, in_maps1,
                                           core_ids=list(range(N_CORES)),
                                           trace=PROFILE)

    t2 = np.zeros((T2_ROWS, T2_COLS), np.float32)
    for c in range(N_CORES):
        t2[c * OWN:(c + 1) * OWN, 0:66] = np.asarray(
            res1.results[c]["t2own"], np.float32).reshape(RANKS, 66)[:OWN]
    t2[T2_PAD, 64] = EL_SMALL
    t2[T2_ISO, 0:64] = b2
    t2[T2_ISO, 64] = EL_BIG
    t2 = _bf(t2)

    nc2 = build_launch2(J, idx2_cols[0].shape[1], er2_cols[0].shape[1],
                        calls2, ecalls2)
    in_maps2 = [dict(T2=t2, idx2=idx2_cols[c], eridx2=er2_cols[c])
                for c in range(N_CORES)]
    res2 = bass_utils.run_bass_kernel_spmd(nc2, in_maps2,
                                           core_ids=list(range(N_CORES)),
                                           trace=PROFILE)
    global LAST_EXEC_NS
    LAST_EXEC_NS = [res1.exec_time_ns, res2.exec_time_ns]

    out = np.zeros((N_NODES, O2), np.float32)
    for c in range(N_CORES):
        r = np.asarray(res2.results[c]["out"], np.float32).reshape(RANKS, O2)
        out[c * OWN + cores[c]["order"]] = r[:OWN]
    return out

